# revision 9
# baseline (speedup 1.0000x reference)
"""BlendShapes model kernel for 8 Trainium2 NeuronCores.

Computation (reference):
    pose_repr = pose[:, 1:].reshape(B, 23, 9) - eye      # (B, J, 9)
    per-joint MLP 9 -> 18 -> 32 -> 8 (ReLU between)      # coff (B, J, 8)
    basis_full = basis[:, None] * mask[:, :, None, None]  # (V, J, 8, 3)
    res = einsum('bjk,vjkc->bvc', coff, basis_full)       # (B, V, 3)

Mapping:
  - Vertices are sharded across the 8 cores (V=6890 padded to 8*864=6912).
  - Each core computes the full MLP with activations laid out transposed
    ([features, batch]) so the final coefficients coff^T [J*8, B] feed the
    big matmul's stationary operand directly — no on-chip transposes.
  - Joints are processed in chunks of 4 (3 for the tail) with block-diagonal
    weights packed on the host, so each MLP layer chunk is one PE matmul.
  - The output (B x Vc*3 slice per core) is PSUM-accumulated over K = 184
    (split 128 + 56), evacuated via ACT/DVE, and streamed to HBM per b-tile.
"""

import numpy as np

N_VERT, N_JOINT, BPJ, BATCH = 6890, 23, 8, 1024
VPAD = 6912  # 8 * 864
VC = VPAD // 8  # 864 vertices per core
VC3 = VC * 3  # 2592
NT = 432  # main matmul N tile (6 uniform tiles of VC3)
NB = BATCH // 128  # 8 b-tiles

# Unified joint chunking: the same joint groups for all three MLP layers so
# every matmul's rhs is an entire [K, :] tile (base partition 0).
# Per chunk of nj joints: L1 [9nj -> 18nj], L2 [18nj -> 32nj], L3 [32nj -> 8nj].
CHUNKS = [(0, 4), (4, 8), (8, 12), (12, 16), (16, 20), (20, 23)]
NCH = len(CHUNKS)

def _offsets(mpj):
    offs, col = [], 0
    for js, je in CHUNKS:
        offs.append(col)
        col += (je - js) * mpj
    return offs, col

W1_OFF, W1_TOT = _offsets(18)  # 414
W2_OFF, W2_TOT = _offsets(32)  # 736
W3_OFF, W3_TOT = _offsets(8)   # 184
W2_OFF = [W1_TOT + o for o in W2_OFF]
W3_OFF = [W1_TOT + W2_TOT + o for o in W3_OFF]
W_COLS = W1_TOT + W2_TOT + W3_TOT  # 1334

# bias_all columns: [0:6] L1 bias per chunk, [6:12] L2 bias per chunk,
# [12:18] L3 bias per chunk, [18:24] eye chunks.
BIAS_COLS = 24

_CACHED = {}


def _build_nc():
    import concourse.tile as tile
    from concourse import bacc, mybir
    from contextlib import ExitStack

    dt = mybir.dt
    f32, f32r, bf16 = dt.float32, dt.float32r, dt.bfloat16
    AF = mybir.ActivationFunctionType
    ALU = mybir.AluOpType

    nc = bacc.Bacc(None, target_bir_lowering=False)

    pose_t = nc.dram_tensor("pose_t", [207, BATCH], f32r, kind="ExternalInput")
    basis_t = nc.dram_tensor("basis_t", [BPJ, VC3], f32r, kind="ExternalInput")
    mask3 = nc.dram_tensor("mask3", [N_JOINT, VC3], bf16, kind="ExternalInput")
    w_all = nc.dram_tensor("w_all", [128, W_COLS], f32r, kind="ExternalInput")
    bias_all = nc.dram_tensor("bias_all", [128, BIAS_COLS], f32, kind="ExternalInput")
    res = nc.dram_tensor("res", [BATCH, VC3], f32, kind="ExternalOutput")

    with ExitStack() as ctx:
        tc = ctx.enter_context(tile.TileContext(nc))
        const = ctx.enter_context(tc.tile_pool(name="const", bufs=1))
        work = ctx.enter_context(tc.tile_pool(name="work", bufs=1))
        outp = ctx.enter_context(tc.tile_pool(name="outp", bufs=2))
        pmlp = ctx.enter_context(tc.tile_pool(name="pmlp", bufs=2, space="PSUM"))
        pmain = ctx.enter_context(tc.tile_pool(name="pmain", bufs=4, space="PSUM"))

        # ---- basis_full path: replicate basis over joints / mask over (k,c)
        # with 0-stride-broadcast DMA reads, multiply on GPSIMD (keeps DVE/ACT
        # free for the PSUM epilogues).
        bf_a = work.tile([128, VC3], f32r, tag="bf_a")
        bf_b = work.tile([56, VC3], f32r, tag="bf_b")
        mk_a = work.tile([128, VC3], bf16, tag="mk_a")
        mk_b = work.tile([56, VC3], bf16, tag="mk_b")
        nc.gpsimd.dma_start(out=bf_a[:], in_=basis_t[:, :].partition_broadcast(16))
        nc.gpsimd.dma_start(
            out=mk_a[:], in_=mask3[0:16, :][:, None, :].broadcast_to([16, BPJ, VC3])
        )
        nc.gpsimd.dma_start(out=bf_b[:], in_=basis_t[:, :].partition_broadcast(7))
        nc.gpsimd.dma_start(
            out=mk_b[:], in_=mask3[16:23, :][:, None, :].broadcast_to([7, BPJ, VC3])
        )

        # ---- constants / pose input (sync queue, in dependency order)
        bias_sb = const.tile([128, BIAS_COLS], f32, tag="bias")
        nc.sync.dma_start(out=bias_sb[:], in_=bias_all[:, :])
        w_sb = const.tile([128, W_COLS], f32r, tag="w")
        nc.sync.dma_start(out=w_sb[:], in_=w_all[:, :])

        pose_c = []
        for c, (js, je) in enumerate(CHUNKS):
            K = 9 * (je - js)
            t = work.tile([K, BATCH], f32r, tag=f"pose_{c}", name=f"pose_{c}")
            nc.sync.dma_start(out=t[:], in_=pose_t[9 * js : 9 * js + K, :])
            pose_c.append(t)

        # pose_repr = pose - eye (in place, per-partition scalar from bias_sb)
        for c, (js, je) in enumerate(CHUNKS):
            K = 9 * (je - js)
            nc.vector.tensor_scalar(
                out=pose_c[c][:],
                in0=pose_c[c][:],
                scalar1=bias_sb[0:K, 18 + c : 19 + c],
                scalar2=None,
                op0=ALU.subtract,
            )

        # basis_full = basis_rep * mask_rep on GPSIMD, tiled so the first
        # main-matmul rhs tiles are ready early.
        for t in range(VC3 // NT):
            sl = slice(t * NT, (t + 1) * NT)
            nc.gpsimd.tensor_tensor(
                out=bf_a[:, sl], in0=bf_a[:, sl], in1=mk_a[:, sl], op=ALU.mult
            )
            nc.gpsimd.tensor_tensor(
                out=bf_b[:, sl], in0=bf_b[:, sl], in1=mk_b[:, sl], op=ALU.mult
            )

        coffT_a = work.tile([128, BATCH], f32r, tag="coffT_a")
        coffT_b = work.tile([56, BATCH], f32r, tag="coffT_b")
        h1 = {}
        h2 = {}
        coff_c = {}

        def mlp_half(h):
            hs = slice(h * 512, (h + 1) * 512)
            # L1: 9nj -> 18nj, ReLU(x + b) on ACT
            for c, (js, je) in enumerate(CHUNKS):
                nj = je - js
                K, M = 9 * nj, 18 * nj
                off = W1_OFF[c]
                ps = pmlp.tile([M, 512], f32, tag="psmlp", name=f"ps1_{c}_{h}")
                nc.tensor.matmul(
                    ps[:],
                    lhsT=w_sb[0:K, off : off + M],
                    rhs=pose_c[c][:, hs],
                    start=True,
                    stop=True,
                )
                if h == 0:
                    h1[c] = work.tile([M, BATCH], f32r, tag=f"h1_{c}", name=f"h1_{c}")
                nc.scalar.activation(
                    h1[c][:, hs], ps[:], AF.Relu, bias=bias_sb[0:M, c : c + 1]
                )
            # L2: 18nj -> 32nj, ReLU on ACT
            for c, (js, je) in enumerate(CHUNKS):
                nj = je - js
                K, M = 18 * nj, 32 * nj
                off = W2_OFF[c]
                ps = pmlp.tile([M, 512], f32, tag="psmlp", name=f"ps2_{c}_{h}")
                nc.tensor.matmul(
                    ps[:],
                    lhsT=w_sb[0:K, off : off + M],
                    rhs=h1[c][:, hs],
                    start=True,
                    stop=True,
                )
                if h == 0:
                    h2[c] = work.tile([M, BATCH], f32r, tag=f"h2_{c}", name=f"h2_{c}")
                nc.scalar.activation(
                    h2[c][:, hs], ps[:], AF.Relu, bias=bias_sb[0:M, 6 + c : 7 + c]
                )
            # L3: 32nj -> 8nj, bias-add only (DVE), into per-chunk coff tiles;
            # small SBUF->SBUF DMAs then merge them into coffT_a / coffT_b
            # (DMA is the only engine that can shift partition bases).
            for c, (js, je) in enumerate(CHUNKS):
                nj = je - js
                K, M = 32 * nj, 8 * nj
                off = W3_OFF[c]
                ps = pmlp.tile([M, 512], f32, tag="psmlp", name=f"ps3_{c}_{h}")
                nc.tensor.matmul(
                    ps[:],
                    lhsT=w_sb[0:K, off : off + M],
                    rhs=h2[c][:, hs],
                    start=True,
                    stop=True,
                )
                if h == 0:
                    coff_c[c] = work.tile(
                        [M, BATCH], f32r, tag=f"coff_{c}", name=f"coff_{c}"
                    )
                nc.vector.tensor_scalar(
                    out=coff_c[c][:, hs],
                    in0=ps[:],
                    scalar1=bias_sb[0:M, 12 + c : 13 + c],
                    scalar2=None,
                    op0=ALU.add,
                )
                if c < 4:
                    dst = coffT_a[32 * c : 32 * c + M, hs]
                else:
                    r0 = 32 * (c - 4)
                    dst = coffT_b[r0 : r0 + M, hs]
                nc.sync.dma_start(out=dst, in_=coff_c[c][:, hs])

        def main_btile(bt):
            bsl = slice(bt * 128, (bt + 1) * 128)
            ostrip = outp.tile([128, VC3], f32, tag="ostrip", name=f"ostrip_{bt}")
            for t in range(VC3 // NT):
                sl = slice(t * NT, (t + 1) * NT)
                ps = pmain.tile([128, NT], f32, tag="ps", name=f"ps_{bt}_{t}")
                nc.tensor.matmul(
                    ps[:],
                    lhsT=coffT_a[:, bsl],
                    rhs=bf_a[:, sl],
                    start=True,
                    stop=False,
                )
                nc.tensor.matmul(
                    ps[:],
                    lhsT=coffT_b[:, bsl],
                    rhs=bf_b[:, sl],
                    start=False,
                    stop=True,
                )
                # evacuate PSUM -> SBUF, split between ACT and DVE
                if (bt * (VC3 // NT) + t) % 2 == 0:
                    nc.scalar.copy(ostrip[:, sl], ps[:])
                else:
                    nc.vector.tensor_copy(ostrip[:, sl], ps[:])
            nc.sync.dma_start(out=res[bsl, :], in_=ostrip[:])

        # First batch-half of the MLP, then its 4 output b-tiles (overlapping
        # the second half's MLP epilogues), then the rest.
        mlp_half(0)
        for bt in range(4):
            main_btile(bt)
        mlp_half(1)
        for bt in range(4, NB):
            main_btile(bt)

    nc.finalize()
    return nc


def _pack_host(pose, basis, mask, w1, b1, w2, b2, w3, b3):
    import ml_dtypes

    pose_t = np.ascontiguousarray(
        pose[:, 1:].reshape(BATCH, 207).T, dtype=np.float32
    )  # [207, B] rows are (j, i)

    basis_t = np.zeros((BPJ, VPAD * 3), np.float32)  # [k, (v, c)]
    basis_t[:, : N_VERT * 3] = basis.transpose(1, 0, 2).reshape(BPJ, N_VERT * 3)

    mask3 = np.zeros((N_JOINT, VPAD * 3), ml_dtypes.bfloat16)  # [j, (v, c)]
    mask3[:, : N_VERT * 3] = np.repeat(mask.T, 3, axis=1).astype(ml_dtypes.bfloat16)

    w_all = np.zeros((128, W_COLS), np.float32)
    bias_all = np.zeros((128, BIAS_COLS), np.float32)
    for (js, je), o1, o2, o3 in zip(CHUNKS, W1_OFF, W2_OFF, W3_OFF):
        for t, j in enumerate(range(js, je)):
            w_all[t * 9 : (t + 1) * 9, o1 + t * 18 : o1 + (t + 1) * 18] = w1[j]
            w_all[t * 18 : (t + 1) * 18, o2 + t * 32 : o2 + (t + 1) * 32] = w2[j]
            w_all[t * 32 : (t + 1) * 32, o3 + t * 8 : o3 + (t + 1) * 8] = w3[j]
    for c, (js, je) in enumerate(CHUNKS):
        nj = je - js
        bias_all[0 : 18 * nj, c] = b1[js:je].reshape(-1)
        bias_all[0 : 32 * nj, 6 + c] = b2[js:je].reshape(-1)
        bias_all[0 : 8 * nj, 12 + c] = b3[js:je].reshape(-1)
        # eye vector for this chunk's pose rows: 1.0 at i in {0, 4, 8}
        ev = np.zeros((nj, 9), np.float32)
        ev[:, [0, 4, 8]] = 1.0
        bias_all[0 : 9 * nj, 18 + c] = ev.reshape(-1)

    return pose_t, basis_t, mask3, w_all, bias_all


def _in_maps(pose, basis, mask, w1, b1, w2, b2, w3, b3):
    pose_t, basis_t, mask3, w_all, bias_all = _pack_host(
        np.asarray(pose, np.float32),
        np.asarray(basis, np.float32),
        np.asarray(mask, np.float32),
        np.asarray(w1, np.float32),
        np.asarray(b1, np.float32),
        np.asarray(w2, np.float32),
        np.asarray(b2, np.float32),
        np.asarray(w3, np.float32),
        np.asarray(b3, np.float32),
    )
    maps = []
    for i in range(8):
        c0 = i * VC3
        maps.append(
            {
                "pose_t": pose_t,
                "basis_t": np.ascontiguousarray(basis_t[:, c0 : c0 + VC3]),
                "mask3": np.ascontiguousarray(mask3[:, c0 : c0 + VC3]),
                "w_all": w_all,
                "bias_all": bias_all,
            }
        )
    return maps


def kernel(pose, basis, mask, w1, b1, w2, b2, w3, b3):
    from concourse.bass_utils import run_bass_kernel_spmd

    if "nc" not in _CACHED:
        _CACHED["nc"] = _build_nc()
    nc = _CACHED["nc"]

    maps = _in_maps(pose, basis, mask, w1, b1, w2, b2, w3, b3)
    r = run_bass_kernel_spmd(nc, maps, core_ids=list(range(8)))
    out = np.concatenate(
        [m["res"].reshape(BATCH, VC, 3) for m in r.results], axis=1
    )
    return np.ascontiguousarray(out[:, :N_VERT, :])


# revision 10
# speedup vs baseline: 1.1790x; 1.1790x over previous
"""BlendShapes model kernel for 8 Trainium2 NeuronCores.

Computation (reference):
    pose_repr = pose[:, 1:].reshape(B, 23, 9) - eye      # (B, J, 9)
    per-joint MLP 9 -> 18 -> 32 -> 8 (ReLU between)      # coff (B, J, 8)
    basis_full = basis[:, None] * mask[:, :, None, None]  # (V, J, 8, 3)
    res = einsum('bjk,vjkc->bvc', coff, basis_full)       # (B, V, 3)

Mapping:
  - Vertices are sharded across the 8 cores (V=6890 padded to 8*864=6912).
  - Each core computes the full MLP with activations laid out transposed
    ([features, batch]) so the final coefficients coff^T [J*8, B] feed the
    big matmul's stationary operand directly — no on-chip transposes.
  - Joints are processed in chunks of 4 (3 for the tail) with block-diagonal
    weights packed on the host, so each MLP layer chunk is one PE matmul.
  - The output (B x Vc*3 slice per core) is PSUM-accumulated over K = 184
    (split 128 + 56), evacuated via ACT/DVE, and streamed to HBM per b-tile.
"""

import numpy as np

N_VERT, N_JOINT, BPJ, BATCH = 6890, 23, 8, 1024
VPAD = 6912  # 8 * 864
VC = VPAD // 8  # 864 vertices per core
VC3 = VC * 3  # 2592
NT = 432  # main matmul N tile (6 uniform tiles of VC3)
NB = BATCH // 128  # 8 b-tiles

# Unified joint chunking: the same joint groups for all three MLP layers so
# every matmul's rhs is an entire [K, :] tile (base partition 0).
# Per chunk of nj joints: L1 [9nj -> 18nj], L2 [18nj -> 32nj], L3 [32nj -> 8nj].
CHUNKS = [(0, 4), (4, 8), (8, 12), (12, 16), (16, 20), (20, 23)]
NCH = len(CHUNKS)

def _offsets(mpj):
    offs, col = [], 0
    for js, je in CHUNKS:
        offs.append(col)
        col += (je - js) * mpj
    return offs, col

W1_OFF, W1_TOT = _offsets(18)  # 414
W2_OFF, W2_TOT = _offsets(32)  # 736
W3_OFF, W3_TOT = _offsets(8)   # 184
W2_OFF = [W1_TOT + o for o in W2_OFF]
W3_OFF = [W1_TOT + W2_TOT + o for o in W3_OFF]
W_COLS = W1_TOT + W2_TOT + W3_TOT  # 1334

# bias_all columns: [0:6] L1 bias per chunk, [6:12] L2 bias per chunk,
# [12:18] L3 bias per chunk, [18:24] eye chunks.
BIAS_COLS = 24
BSCALE = 8192.0  # 2**13, exact in bf16; descaled exactly in the epilogue
DESCALE = 1.0 / 8192.0

_CACHED = {}


def _build_nc():
    import concourse.tile as tile
    from concourse import bacc, mybir
    from contextlib import ExitStack

    dt = mybir.dt
    f32, f16, bf16 = dt.float32, dt.float16, dt.bfloat16
    AF = mybir.ActivationFunctionType
    ALU = mybir.AluOpType

    nc = bacc.Bacc(None, target_bir_lowering=False)

    pose_t = nc.dram_tensor("pose_t", [207, BATCH], f32, kind="ExternalInput")
    basis_t = nc.dram_tensor("basis_t", [BPJ, VC3], f32, kind="ExternalInput")
    mask3 = nc.dram_tensor("mask3", [N_JOINT, VC3], bf16, kind="ExternalInput")
    w_all = nc.dram_tensor("w_all", [128, W_COLS], f16, kind="ExternalInput")
    bias_all = nc.dram_tensor("bias_all", [128, BIAS_COLS], f32, kind="ExternalInput")
    res = nc.dram_tensor("res", [BATCH, VC3], f32, kind="ExternalOutput")

    with ExitStack() as ctx:
        tc = ctx.enter_context(tile.TileContext(nc))
        const = ctx.enter_context(tc.tile_pool(name="const", bufs=1))
        work = ctx.enter_context(tc.tile_pool(name="work", bufs=1))
        outp = ctx.enter_context(tc.tile_pool(name="outp", bufs=2))
        pmlp = ctx.enter_context(tc.tile_pool(name="pmlp", bufs=2, space="PSUM"))
        pmain = ctx.enter_context(tc.tile_pool(name="pmain", bufs=4, space="PSUM"))

        # ---- basis_full path: replicate basis over joints / mask over (k,c)
        # with 0-stride-broadcast DMA reads, multiply on GPSIMD (keeps DVE/ACT
        # free for the PSUM epilogues).
        bf_a = work.tile([128, VC3], f32, tag="bf_a")
        bf_b = work.tile([56, VC3], f32, tag="bf_b")
        bfm_a = work.tile([128, VC3], f16, tag="bfm_a")
        bfm_b = work.tile([56, VC3], f16, tag="bfm_b")
        mk_a = work.tile([128, VC3], bf16, tag="mk_a")
        mk_b = work.tile([56, VC3], bf16, tag="mk_b")
        nc.gpsimd.dma_start(out=bf_a[:], in_=basis_t[:, :].partition_broadcast(16))
        nc.gpsimd.dma_start(
            out=mk_a[:], in_=mask3[0:16, :][:, None, :].broadcast_to([16, BPJ, VC3])
        )
        nc.gpsimd.dma_start(out=bf_b[:], in_=basis_t[:, :].partition_broadcast(7))
        nc.gpsimd.dma_start(
            out=mk_b[:], in_=mask3[16:23, :][:, None, :].broadcast_to([7, BPJ, VC3])
        )

        # ---- constants / pose input (sync queue, in dependency order)
        bias_sb = const.tile([128, BIAS_COLS], f32, tag="bias")
        nc.sync.dma_start(out=bias_sb[:], in_=bias_all[:, :])
        w_sb = const.tile([128, W_COLS], f16, tag="w")
        nc.sync.dma_start(out=w_sb[:], in_=w_all[:, :])

        pose_c = []
        for c, (js, je) in enumerate(CHUNKS):
            K = 9 * (je - js)
            t = work.tile([K, BATCH], f32, tag=f"praw_{c}", name=f"praw_{c}")
            nc.sync.dma_start(out=t[:], in_=pose_t[9 * js : 9 * js + K, :])
            pose_c.append(t)

        # pose_repr = pose - eye; converts f32 -> f16 on the way out
        for c, (js, je) in enumerate(CHUNKS):
            K = 9 * (je - js)
            t16 = work.tile([K, BATCH], f16, tag=f"pose_{c}", name=f"pose_{c}")
            nc.vector.tensor_scalar(
                out=t16[:],
                in0=pose_c[c][:],
                scalar1=bias_sb[0:K, 18 + c : 19 + c],
                scalar2=None,
                op0=ALU.subtract,
            )
            pose_c[c] = t16

        # basis_full = basis_rep * mask_rep on GPSIMD, tiled so the first
        # main-matmul rhs tiles are ready early.
        for t in range(VC3 // NT):
            sl = slice(t * NT, (t + 1) * NT)
            nc.gpsimd.tensor_tensor(
                out=bfm_a[:, sl], in0=bf_a[:, sl], in1=mk_a[:, sl], op=ALU.mult
            )
            nc.gpsimd.tensor_tensor(
                out=bfm_b[:, sl], in0=bf_b[:, sl], in1=mk_b[:, sl], op=ALU.mult
            )

        coffT_a = work.tile([128, BATCH], f16, tag="coffT_a")
        coffT_b = work.tile([56, BATCH], f16, tag="coffT_b")
        h1 = {}
        h2 = {}
        coff_c = {}

        def mlp_half(h):
            hs = slice(h * 512, (h + 1) * 512)
            # L1: 9nj -> 18nj, ReLU(x + b) on ACT
            for c, (js, je) in enumerate(CHUNKS):
                nj = je - js
                K, M = 9 * nj, 18 * nj
                off = W1_OFF[c]
                ps = pmlp.tile([M, 512], f32, tag="psmlp", name=f"ps1_{c}_{h}")
                nc.tensor.matmul(
                    ps[:],
                    lhsT=w_sb[0:K, off : off + M],
                    rhs=pose_c[c][:, hs],
                    start=True,
                    stop=True,
                )
                if h == 0:
                    h1[c] = work.tile([M, BATCH], f16, tag=f"h1_{c}", name=f"h1_{c}")
                nc.scalar.activation(
                    h1[c][:, hs], ps[:], AF.Relu, bias=bias_sb[0:M, c : c + 1]
                )
            # L2: 18nj -> 32nj, ReLU on ACT
            for c, (js, je) in enumerate(CHUNKS):
                nj = je - js
                K, M = 18 * nj, 32 * nj
                off = W2_OFF[c]
                ps = pmlp.tile([M, 512], f32, tag="psmlp", name=f"ps2_{c}_{h}")
                nc.tensor.matmul(
                    ps[:],
                    lhsT=w_sb[0:K, off : off + M],
                    rhs=h1[c][:, hs],
                    start=True,
                    stop=True,
                )
                if h == 0:
                    h2[c] = work.tile([M, BATCH], f16, tag=f"h2_{c}", name=f"h2_{c}")
                nc.scalar.activation(
                    h2[c][:, hs], ps[:], AF.Relu, bias=bias_sb[0:M, 6 + c : 7 + c]
                )
            # L3: 32nj -> 8nj, bias-add only (DVE), into per-chunk coff tiles;
            # small SBUF->SBUF DMAs then merge them into coffT_a / coffT_b
            # (DMA is the only engine that can shift partition bases).
            for c, (js, je) in enumerate(CHUNKS):
                nj = je - js
                K, M = 32 * nj, 8 * nj
                off = W3_OFF[c]
                ps = pmlp.tile([M, 512], f32, tag="psmlp", name=f"ps3_{c}_{h}")
                nc.tensor.matmul(
                    ps[:],
                    lhsT=w_sb[0:K, off : off + M],
                    rhs=h2[c][:, hs],
                    start=True,
                    stop=True,
                )
                if h == 0:
                    coff_c[c] = work.tile(
                        [M, BATCH], f16, tag=f"coff_{c}", name=f"coff_{c}"
                    )
                nc.vector.tensor_scalar(
                    out=coff_c[c][:, hs],
                    in0=ps[:],
                    scalar1=bias_sb[0:M, 12 + c : 13 + c],
                    scalar2=None,
                    op0=ALU.add,
                )
                if c < 4:
                    dst = coffT_a[32 * c : 32 * c + M, hs]
                else:
                    r0 = 32 * (c - 4)
                    dst = coffT_b[r0 : r0 + M, hs]
                nc.gpsimd.dma_start(out=dst, in_=coff_c[c][:, hs])

        def main_btile(bt):
            bsl = slice(bt * 128, (bt + 1) * 128)
            ostrip = outp.tile([128, VC3], f32, tag="ostrip", name=f"ostrip_{bt}")
            for t in range(VC3 // NT):
                sl = slice(t * NT, (t + 1) * NT)
                ps = pmain.tile([128, NT], f32, tag="ps", name=f"ps_{bt}_{t}")
                nc.tensor.matmul(
                    ps[:],
                    lhsT=coffT_a[:, bsl],
                    rhs=bfm_a[:, sl],
                    start=True,
                    stop=False,
                )
                nc.tensor.matmul(
                    ps[:],
                    lhsT=coffT_b[:, bsl],
                    rhs=bfm_b[:, sl],
                    start=False,
                    stop=True,
                )
                # evacuate PSUM -> SBUF, split between ACT and DVE
                if (bt * (VC3 // NT) + t) % 2 == 0:
                    nc.scalar.activation(
                        ostrip[:, sl], ps[:], AF.Copy, scale=DESCALE
                    )
                else:
                    nc.vector.tensor_scalar(
                        out=ostrip[:, sl],
                        in0=ps[:],
                        scalar1=DESCALE,
                        scalar2=None,
                        op0=ALU.mult,
                    )
            nc.sync.dma_start(out=res[bsl, :], in_=ostrip[:])

        # First batch-half of the MLP, then its 4 output b-tiles (overlapping
        # the second half's MLP epilogues), then the rest.
        mlp_half(0)
        for bt in range(4):
            main_btile(bt)
        mlp_half(1)
        for bt in range(4, NB):
            main_btile(bt)

    nc.finalize()
    return nc


def _pack_host(pose, basis, mask, w1, b1, w2, b2, w3, b3):
    import ml_dtypes

    pose_t = np.ascontiguousarray(
        pose[:, 1:].reshape(BATCH, 207).T, dtype=np.float32
    )  # [207, B] rows are (j, i)

    basis_t = np.zeros((BPJ, VPAD * 3), np.float32)  # [k, (v, c)]
    basis_t[:, : N_VERT * 3] = basis.transpose(1, 0, 2).reshape(BPJ, N_VERT * 3)

    mask3 = np.zeros((N_JOINT, VPAD * 3), ml_dtypes.bfloat16)  # [j, (v, c)]
    mask3[:, : N_VERT * 3] = (np.repeat(mask.T, 3, axis=1) * BSCALE).astype(
        ml_dtypes.bfloat16
    )

    w_all = np.zeros((128, W_COLS), np.float16)
    bias_all = np.zeros((128, BIAS_COLS), np.float32)
    for (js, je), o1, o2, o3 in zip(CHUNKS, W1_OFF, W2_OFF, W3_OFF):
        for t, j in enumerate(range(js, je)):
            w_all[t * 9 : (t + 1) * 9, o1 + t * 18 : o1 + (t + 1) * 18] = w1[j]
            w_all[t * 18 : (t + 1) * 18, o2 + t * 32 : o2 + (t + 1) * 32] = w2[j]
            w_all[t * 32 : (t + 1) * 32, o3 + t * 8 : o3 + (t + 1) * 8] = w3[j]
    for c, (js, je) in enumerate(CHUNKS):
        nj = je - js
        bias_all[0 : 18 * nj, c] = b1[js:je].reshape(-1)
        bias_all[0 : 32 * nj, 6 + c] = b2[js:je].reshape(-1)
        bias_all[0 : 8 * nj, 12 + c] = b3[js:je].reshape(-1)
        # eye vector for this chunk's pose rows: 1.0 at i in {0, 4, 8}
        ev = np.zeros((nj, 9), np.float32)
        ev[:, [0, 4, 8]] = 1.0
        bias_all[0 : 9 * nj, 18 + c] = ev.reshape(-1)

    return pose_t, basis_t, mask3, w_all, bias_all


def _in_maps(pose, basis, mask, w1, b1, w2, b2, w3, b3):
    pose_t, basis_t, mask3, w_all, bias_all = _pack_host(
        np.asarray(pose, np.float32),
        np.asarray(basis, np.float32),
        np.asarray(mask, np.float32),
        np.asarray(w1, np.float32),
        np.asarray(b1, np.float32),
        np.asarray(w2, np.float32),
        np.asarray(b2, np.float32),
        np.asarray(w3, np.float32),
        np.asarray(b3, np.float32),
    )
    maps = []
    for i in range(8):
        c0 = i * VC3
        maps.append(
            {
                "pose_t": pose_t,
                "basis_t": np.ascontiguousarray(basis_t[:, c0 : c0 + VC3]),
                "mask3": np.ascontiguousarray(mask3[:, c0 : c0 + VC3]),
                "w_all": w_all,
                "bias_all": bias_all,
            }
        )
    return maps


def kernel(pose, basis, mask, w1, b1, w2, b2, w3, b3):
    from concourse.bass_utils import run_bass_kernel_spmd

    if "nc" not in _CACHED:
        _CACHED["nc"] = _build_nc()
    nc = _CACHED["nc"]

    maps = _in_maps(pose, basis, mask, w1, b1, w2, b2, w3, b3)
    r = run_bass_kernel_spmd(nc, maps, core_ids=list(range(8)))
    out = np.concatenate(
        [m["res"].reshape(BATCH, VC, 3) for m in r.results], axis=1
    )
    return np.ascontiguousarray(out[:, :N_VERT, :])
